# revision 63
# baseline (speedup 1.0000x reference)
"""Complex coherency loss, distributed over 8 TRN2 NeuronCores.

Data-parallel over batch: core b computes the partial coherency sum for
batch element b; the host sums the 8x128 partials and finishes the mean.

Layout ("parity"): the host reorders each [C=64, L=16384] shard into
[P=128, N=8192] with partition p = 2c + (l % 2) and free n = l // 2, so
free column n covers the position PAIR (2n, 2n+1). The four reordered
tensors are packed group-major into ONE bf16 DRAM tensor
X[:, 4*col_g : 4*col_g + 4*fd] = [pr_g | pi_g | tr_g | ti_g], so each
compute group needs exactly one HWDGE load (big contiguous lines; the
f32->bf16 cast happens on host — the on-device casting path is
SWDGE-only and tops out ~200 GB/s aggregate, which capped the old
kernel at ~96us).

Per-core pipeline:
  - sync-queue HWDGE DMAs stream bf16 inputs (1 DMA per group)
  - DVE: 4 cross products + 2 squares (bf16 2x), ACT: 2 squares
  - PE : per 512-chunk, 8 matmuls against [128, 8] +/-1 selector weights
         channel-reduce everything into one [8, 2048] PSUM tile
         (row r = 2q + parity: q in {ptr, pti, pa, ta})
  - ACT copies PSUM -> SBUF; one contiguous DMA appends to a [8, 8196]
    DRAM staging buffer (stg[2q+par, n] = channel sum of quantity q at
    position l = 2n + par)
  - Tail in 3 chunks (rows 0-63 / 64-117 / 118-127 of the halo view,
    triggered as soon as the staging columns they read are resident) so
    only the last 10-row chunk sits on the serial critical path after
    the final group. Per chunk: ONE strided DMA loads all 8 staging
    rows into a [cnt, 8*68] halo tile (partition p' holds pair index
    n = 64*(row0+p') + i); the k=5 window sums come from shared
    partials (4 adds instead of 8):
      S2[n] = E[n]+O[n];  T[n] = S2[n]+S2[n+1]
      win[2n]   = T[n]   + E[n+2]
      win[2n+1] = T[n+1] + O[n]
    then ratio = (wr^2+wi^2)/(wa*wt) for both parities in one pass,
    masked at the 2 invalid trailing pairs, sqrt+free-axis-accumulated
    on ACT (summing both parities) and DMA'd to out[128, 1].
  Staging/halo DMAs ride the sync HWDGE ring; out/init DMAs ride the
  scalar HWDGE ring (keeps them out of the halo loads' FIFO; gpsimd
  SWDGE is avoided entirely — it adds a ~7us queue-drain epilogue).
"""

import numpy as np
import ml_dtypes

import concourse.bass as bass
import concourse.bacc as bacc
import concourse.mybir as mybir
import concourse.tile as tile
from concourse.bass_utils import run_bass_kernel_spmd

B, C, L = 8, 64, 16384
K = 5
P = 128
N = (C * L) // P          # 8192 free positions per core view (pairs)
NVALID = L - K + 1        # 16380
FD = 2048                 # max free columns per compute group
CH = 512                  # matmul moving-dim chunk
STG_W = N + 4             # staging row width (4 zero-pad columns)

GROUP_FDS = [512, 1536, 2048, 2048, 1536, 512]
assert sum(GROUP_FDS) == N
# Square-pass engine per (group, which): DVE/ACT split tuned so
# DVE ~= ACT ~= 28-29us busy (DVE also owns the 4 cross-product passes
# and gets 37.5% of the square columns, spread across the stream so
# neither engine starves the PE late; ACT owns the PSUM drains).
SQ_ENG = {0: "aa", 1: "aa", 2: "av", 3: "av", 4: "av", 5: "av"}

F32 = mybir.dt.float32
BF16 = mybir.dt.bfloat16

PROFILE = False
TRACE_DIR = None
LAST_RESULT = None


def _selector_weights() -> np.ndarray:
    """Five [128, 8] weight matrices, packed as [128, 40] bf16.

    Matrix w maps a product tensor into PSUM rows 2q+par (par = p % 2):
      w=0: m1,m2 -> rows 0,1 (ptr, +)    w=1: m3 -> rows 2,3 (pti, +)
      w=2: m4    -> rows 2,3 (pti, -)    w=3: s1,s2 -> rows 4,5 (pa, +)
      w=4: s3,s4 -> rows 6,7 (ta, +)
    """
    w = np.zeros((P, 5 * 8), dtype=np.float32)
    p = np.arange(P)
    h = p % 2
    w[p, 0 * 8 + 0 + h] = 1.0
    w[p, 1 * 8 + 2 + h] = 1.0
    w[p, 2 * 8 + 2 + h] = -1.0
    w[p, 3 * 8 + 4 + h] = 1.0
    w[p, 4 * 8 + 6 + h] = 1.0
    return w.astype(ml_dtypes.bfloat16)


def build_nc() -> bacc.Bacc:
    nc = bacc.Bacc("TRN2", target_bir_lowering=False, debug=False)

    x_d = nc.dram_tensor("x", [P, 4 * N], BF16, kind="ExternalInput").ap()
    out_d = nc.dram_tensor("out", [1, 1], F32, kind="ExternalOutput").ap()
    w_d = nc.inline_tensor(_selector_weights(), name="selw").ap()

    with tile.TileContext(nc) as tc:
        with (
            tc.tile_pool(name="consts", bufs=1) as consts,
            tc.tile_pool(name="ins", bufs=3) as ins,
            tc.tile_pool(name="prods", bufs=2) as prods,
            tc.tile_pool(name="drains", bufs=2) as drains,
            tc.tile_pool(name="fin", bufs=1) as fin,
            tc.tile_pool(name="psum", bufs=2, space="PSUM") as psum,
            tc.tile_pool(name="dram", bufs=1, space="DRAM") as dram,
        ):
            w_sb = consts.tile([P, 5 * 8], BF16)
            nc.sync.dma_start(w_sb[:, :], w_d)

            # Input tiles first: queue the group loads as early as
            # possible (a ring entry stalls until its buffer frees, so
            # deep queuing is safe and keeps the ring fed).
            xts = []
            col = 0
            for g, fd in enumerate(GROUP_FDS):
                xt = ins.tile([P, 4 * fd], BF16, name="xt", tag="xt",
                              padded_shape=[P, 4 * FD])
                nc.sync.dma_start(xt[:, :], x_d[:, 4 * col:4 * col + 4 * fd])
                xts.append(xt)
                col += fd

            # DRAM staging: row r = 2q + parity, column n holds the
            # channel sum of quantity q at position l = 2n + parity.
            # DRAM (not SBUF) because the tail's transposing halo
            # gather then lowers to ONE dma_start with a free-form 3D
            # AP (~0.8us issue); SBUF sources pin the partition dim to
            # dim0, forcing 8 per-row DMAs at ~1-1.8us of sequencer
            # issue time each.
            stg = dram.tile([8, STG_W], F32)

            # Zero the staging tail so halo reads past N are defined.
            zeros = consts.tile([1, 8 * (STG_W - N)], F32)
            nc.vector.memset(zeros[:, :], 0.0)
            nc.scalar.dma_start(stg[:, N:STG_W], zeros[:, :])

            # Pre-warm the Sqrt activation table so the lazy table load
            # doesn't land on the serial tail.
            warm = consts.tile([P, 1], F32)
            nc.vector.memset(warm[:, :], 1.0)
            nc.scalar.sqrt(warm[:, :], warm[:, :])

            # Validity mask for the last tail chunk (rows 94-127):
            # pairs n = 8190, 8191 (positions l >= 16380) sit at
            # [p'=33, i=62..63] for BOTH parities (free = par*64 + i).
            mask_b = consts.tile([34, 128], F32)
            nc.vector.memset(mask_b[:, :], 1.0)
            nc.scalar.dma_start(mask_b[33:34, 62:64], zeros[0:1, 0:2])
            nc.scalar.dma_start(mask_b[33:34, 126:128], zeros[0:1, 0:2])

            ones = consts.tile([P, 1], F32)
            nc.vector.memset(ones[:, :], 1.0)

            # Order interleaves the DVE-produced planes (m12, m34, and
            # usually sqt) with the ACT-produced sqp so the PE never
            # waits long on a single producer.
            mm_plan = [  # (weight idx, product slot, start, stop)
                (0, 0, True, False),   # m1 = pr*tr
                (0, 1, False, False),  # m2 = pi*ti
                (3, 4, False, False),  # s1 = pr^2
                (3, 5, False, False),  # s2 = pi^2
                (1, 2, False, False),  # m3 = pi*tr
                (2, 3, False, False),  # m4 = pr*ti (negative weights)
                (4, 6, False, False),  # s3 = tr^2
                (4, 7, False, True),   # s4 = ti^2
            ]

            def tail_win(kc, row0, cnt, eng):
                """Halo gather + window sums for rows [row0, row0+cnt).

                Halo partition p' holds pair indices n = 64*(row0+p')+i,
                i in [0, 68). One DMA gathers all 8 staging rows. `eng`
                does the adds: gpsimd (otherwise idle) for the
                overlapped chunk so DVE stays on the main loop. The
                ratio part is emitted separately AFTER the main loop —
                engines run their programs in order, so a mid-loop
                DVE instruction that waits on gpsimd results would
                stall the product passes behind it (measured 9.3us).
                """
                h = fin.tile([cnt, 8 * 68], F32, name=f"halo{kc}",
                             tag=f"halo{kc}")
                # ONE transposing gather from DRAM staging: dest
                # partition p' reads all 8 staging rows at columns
                # 64*(row0+p') .. +68.
                src = bass.AP(
                    tensor=stg.tensor,
                    offset=stg.offset + 64 * row0,
                    ap=[[64, cnt], [STG_W, 8], [1, 68]],
                )
                nc.scalar.dma_start(
                    h.rearrange("p (r i) -> p r i", r=8), src)
                h4 = h.rearrange("p (q par i) -> p q par i", q=4, par=2)
                hE = h4[:, :, 0, :]          # [cnt, 4, 68]
                hO = h4[:, :, 1, :]

                # Shared-partial window sums (4 adds instead of 8):
                s2 = fin.tile([cnt, 4 * 66], F32, name=f"s2_{kc}",
                              tag=f"s2_{kc}")
                s2r = s2.rearrange("p (q i) -> p q i", q=4)
                eng.tensor_add(s2r, hE[:, :, 0:66], hO[:, :, 0:66])
                tt = fin.tile([cnt, 4 * 65], F32, name=f"tt_{kc}",
                              tag=f"tt_{kc}")
                ttr = tt.rearrange("p (q i) -> p q i", q=4)
                eng.tensor_add(ttr, s2r[:, :, 0:65], s2r[:, :, 1:66])

                # w_all free layout: par-major (par, q, 64)
                w_all = fin.tile([cnt, 2 * 4 * 64], F32, name=f"wa_{kc}",
                                 tag=f"wa_{kc}")
                wv = w_all.rearrange("p (par q f) -> p par q f", par=2, q=4)
                # win[2n]   = T[n]   + E[n+2]
                eng.tensor_add(
                    wv[:, 0, :, :], ttr[:, :, 0:64], hE[:, :, 2:66])
                # win[2n+1] = T[n+1] + O[n]
                eng.tensor_add(
                    wv[:, 1, :, :], ttr[:, :, 1:65], hO[:, :, 0:64])
                return wv

            def tail_ratio(kc, row0, cnt, wv, eng):
                """Per-window |coherency| + accumulate -> acc [cnt, 1].

                eng=gpsimd variant routes the divide through ACT's
                Sqrt/Rsqrt tables (sqrt(n2)*rsqrt(d2)) so DVE is never
                touched; eng=vector uses DVE's fast reciprocal. All ACT
                pieces are emitted after the main loop so ACT's
                in-order program can't stall the drains.
                """
                wr = wv[:, :, 0, :]
                wi = wv[:, :, 1, :]
                wa = wv[:, :, 2, :]
                wt = wv[:, :, 3, :]
                n2 = fin.tile([cnt, 128], F32, name=f"n2_{kc}",
                              tag=f"n2_{kc}")
                t2 = fin.tile([cnt, 128], F32, name=f"t2_{kc}",
                              tag=f"t2_{kc}")
                n2r = n2.rearrange("p (par f) -> p par f", par=2)
                t2r = t2.rearrange("p (par f) -> p par f", par=2)
                eng.tensor_mul(n2r, wr, wr)
                eng.tensor_mul(t2r, wi, wi)
                eng.tensor_add(n2[:, :], n2[:, :], t2[:, :])
                d2 = fin.tile([cnt, 128], F32, name=f"d2_{kc}",
                              tag=f"d2_{kc}")
                d2r = d2.rearrange("p (par f) -> p par f", par=2)
                eng.tensor_mul(d2r, wa, wt)
                acc = fin.tile([cnt, 1], F32, name=f"acc{kc}",
                               tag=f"acc{kc}")
                rd = fin.tile([cnt, 128], F32, name=f"rd_{kc}",
                              tag=f"rd_{kc}")
                nc.vector.reciprocal_approx_fast(rd[:, :], d2[:, :])
                eng.tensor_mul(n2[:, :], n2[:, :], rd[:, :])
                if row0 + cnt == P:
                    eng.tensor_mul(
                        n2[:, :], n2[:, :], mask_b[0:cnt, :])
                sq = fin.tile([cnt, 128], F32, name=f"sq{kc}",
                              tag=f"sq{kc}")
                nc.scalar.activation(
                    sq[:, :], n2[:, :],
                    mybir.ActivationFunctionType.Sqrt,
                    accum_out=acc[:, :],
                )
                return acc

            # trigger col -> (kc, row0, cnt): chunk reads stg cols
            # [64*row0, 64*(row0+cnt) + 68) -> needs staging through
            # trigger col. Chunks A/A2 window on gpsimd fully
            # overlapped; only the 34-row chunk B depends on the final
            # group. Ratio phases are emitted after the loop so no
            # in-order engine program blocks the main stream.
            tails = {4096: (0, 0, 62), 6144: (1, 62, 32)}
            wvs = {}

            col = 0
            for g, fd in enumerate(GROUP_FDS):
                xt = xts[g]
                t_p = xt[:, 0:2 * fd]           # (pr | pi)
                t_t = xt[:, 2 * fd:4 * fd]      # (tr | ti)

                # (pi|pr): block-swapped view of t_p
                t_p_sw = bass.AP(
                    tensor=xt.tensor,
                    offset=xt.offset + fd,
                    ap=[list(xt.ap[0]), [-fd, 2], [1, fd]],
                )
                t_p3 = t_p.rearrange("p (b f) -> p b f", b=2)
                t_t3 = t_t.rearrange("p (b f) -> p b f", b=2)

                # m12 = (pr*tr | pi*ti), m34 = (pi*tr | pr*ti)
                m12 = prods.tile([P, 2 * fd], BF16, name="m12", tag="m12",
                                 padded_shape=[P, 2 * FD])
                m34 = prods.tile([P, 2 * fd], BF16, name="m34", tag="m34",
                                 padded_shape=[P, 2 * FD])
                nc.vector.tensor_mul(
                    m12.rearrange("p (b f) -> p b f", b=2), t_p3, t_t3)
                nc.vector.tensor_mul(
                    m34.rearrange("p (b f) -> p b f", b=2), t_p_sw, t_t3)

                # sqp = (pr^2 | pi^2), sqt = (tr^2 | ti^2); the last two
                # groups square on DVE (idle then) to unclog ACT's queue
                sqp = prods.tile([P, 2 * fd], BF16, name="sqp", tag="sqp",
                                 padded_shape=[P, 2 * FD])
                sqt = prods.tile([P, 2 * fd], BF16, name="sqt", tag="sqt",
                                 padded_shape=[P, 2 * FD])
                for sq, t_in, which in ((sqp, t_p, 0), (sqt, t_t, 1)):
                    e = SQ_ENG[g][which]
                    if e == "v":
                        nc.vector.tensor_mul(sq[:, :], t_in, t_in)
                    elif e == "g":
                        nc.gpsimd.tensor_mul(sq[:, :], t_in, t_in)
                    else:
                        nc.scalar.square(sq[:, :], t_in)

                prod_slices = [
                    m12[:, 0:fd], m12[:, fd:2 * fd],
                    m34[:, 0:fd], m34[:, fd:2 * fd],
                    sqp[:, 0:fd], sqp[:, fd:2 * fd],
                    sqt[:, 0:fd], sqt[:, fd:2 * fd],
                ]

                ps = psum.tile([8, fd], F32, name="ps", tag="ps",
                               padded_shape=[8, FD])
                for widx, pslot, start, stop in mm_plan:
                    prod = prod_slices[pslot]
                    lhsT = w_sb[:, widx * 8:(widx + 1) * 8]
                    for kk in range(0, fd, CH):
                        ks = slice(kk, min(kk + CH, fd))
                        nc.tensor.matmul(
                            ps[:, ks], lhsT, prod[:, ks],
                            start=start, stop=stop,
                        )

                dr = drains.tile([8, fd], F32, name="dr", tag="dr",
                                 padded_shape=[8, FD])
                nc.scalar.activation(
                    dr[:, :], ps[:, :], mybir.ActivationFunctionType.Copy
                )
                nc.sync.dma_start(stg[:, col:col + fd], dr[:, :])
                col += fd

                if col in tails:
                    kc, row0, cnt = tails[col]
                    wvs[(kc, row0, cnt)] = tail_win(kc, row0, cnt,
                                                    nc.gpsimd)

            accs = [(tail_ratio(kc, row0, cnt, wv, nc.gpsimd), cnt)
                    for (kc, row0, cnt), wv in wvs.items()]
            wv_b = tail_win(2, 94, 34, nc.vector)
            accs.append((tail_ratio(2, 94, 34, wv_b, nc.vector), 34))

            # Partition-reduce the accumulators on PE into one scalar
            # so the final DRAM write is ONE 4-byte descriptor — a
            # [cnt, 1] out DMA is cnt scattered 4B descriptors whose
            # write receipts add ~5us to the epilogue barrier.
            psf = psum.tile([1, 1], F32, name="psf", tag="ps",
                            padded_shape=[8, FD])
            for i, (acc, cnt) in enumerate(accs):
                nc.tensor.matmul(psf[:, :], acc[:, :], ones[0:cnt, :],
                                 start=(i == 0),
                                 stop=(i == len(accs) - 1))
            res = fin.tile([1, 1], F32, name="res", tag="res")
            nc.scalar.activation(
                res[:, :], psf[:, :], mybir.ActivationFunctionType.Copy)
            nc.scalar.dma_start(out_d[0:1, 0:1], res[:, :])

    nc.compile()
    return nc


_NC = None


def _get_nc() -> bacc.Bacc:
    global _NC
    if _NC is None:
        _NC = build_nc()
    return _NC


def _parity_view(x: np.ndarray) -> np.ndarray:
    # [64, 16384] -> [128, 8192] with partition 2c + (l%2), free l//2
    return x.reshape(C, N, 2).transpose(0, 2, 1).reshape(P, N)


def _pack_core(pr, pi, tr, ti) -> np.ndarray:
    """Group-major pack: X[:, 4c:4c+4fd] = [pr_g | pi_g | tr_g | ti_g],
    cast to bf16 on host."""
    x = np.empty((P, 4 * N), dtype=ml_dtypes.bfloat16)
    views = [_parity_view(pr), _parity_view(pi),
             _parity_view(tr), _parity_view(ti)]
    col = 0
    for fd in GROUP_FDS:
        off = 4 * col
        for v in views:
            x[:, off:off + fd] = v[:, col:col + fd]
            off += fd
        col += fd
    return x


def kernel(pred_real, pred_imag, targ_real, targ_imag, filter_size=5):
    global LAST_RESULT
    assert int(filter_size) == K
    nc = _get_nc()

    pred_real = np.asarray(pred_real, dtype=np.float32)
    pred_imag = np.asarray(pred_imag, dtype=np.float32)
    targ_real = np.asarray(targ_real, dtype=np.float32)
    targ_imag = np.asarray(targ_imag, dtype=np.float32)

    in_maps = []
    for b in range(B):
        in_maps.append({
            "x": _pack_core(pred_real[b], pred_imag[b],
                            targ_real[b], targ_imag[b]),
        })

    kwargs = {}
    if PROFILE:
        kwargs = dict(trace=True)
        if TRACE_DIR is not None:
            import os
            os.makedirs(TRACE_DIR, exist_ok=True)
            kwargs["tmpdir"] = TRACE_DIR
    res = run_bass_kernel_spmd(nc, in_maps, core_ids=list(range(B)), **kwargs)
    LAST_RESULT = res

    total = 0.0
    for r in res.results:
        total += float(np.asarray(r["out"], dtype=np.float64)[0, 0])
    coh = total / (B * NVALID)
    return np.float32(1.0 - coh)
